# revision 1
# baseline (speedup 1.0000x reference)
"""Trainium2 Bass kernel for nn_BDPool (corner-pool style block).

Per-sample network (NCHW, x: (256,128,128)):
    p1 = relu(bn1(conv3x3_256to128(x)))
    p2 = relu(bn2(conv3x3_256to128(x)))
    pool1 = reverse-cummax_H(p1); pool2 = reverse-cummax_W(p2)
    r  = relu(bn_p(conv3x3_128to256(pool1+pool2)) + bn_c1(conv1x1_256to256(x)))
    out = relu(bn_c2(conv3x3_256to256(r)))

Sharding: data-parallel over batch; core i computes sample i entirely.

Implementation notes:
- BN folded into conv weights (scale) + per-channel bias applied during
  PSUM->SBUF eviction on the scalar engine (Relu activation with bias AP).
- 3x3 convs are 9 shifted fp32r matmuls accumulating in PSUM; inputs live
  in SBUF with 1-pixel zero-padded borders (row stride W+2).
- pool2 (reverse cummax along W) is a 7-step in-place log-shift max per
  strip, overlapped inside the conv phase; strips round-trip via DRAM.
- pool1 (reverse cummax along H) is a 127-step row max-chain emitted
  bottom-up interleaved with the conv strips so it overlaps on DVE.
- Phases are pipelined bottom-up: p1/p2 conv strips -> per-strip s-add ->
  p_conv+conv1 (fused PSUM group) -> final conv, with the intermediate r
  held in a 3-deep SBUF ring (no DRAM round trip).
"""

import numpy as np

import concourse.bass as bass
import concourse.mybir as mybir
from concourse.tile import TileContext
from concourse.bass_utils import run_bass_kernel_spmd

dt = mybir.dt
F32 = dt.float32
F32R = dt.float32r
RELU = mybir.ActivationFunctionType.Relu
MAX = mybir.AluOpType.max
ADD = mybir.AluOpType.add

C = 256
M = 128
W = 128
SH = 8  # strip height
NT = SH // 4

EPS = 1e-5


# ---------------------------------------------------------------------------
# walrus wait-limit workaround: split instructions carrying >1 sem wait (or
# >1 sem update) into a chain of NOPs each carrying one.
_wfix_counter = [0]


def _mk_nop(nc, engine, waits=None, updates=None):
    _wfix_counter[0] += 1
    si = mybir.SyncInfo(on_wait=list(waits or []), on_update=list(updates or []))
    inst = mybir.InstNoOp(
        name=f"WFIX-{_wfix_counter[0]}",
        engine=engine,
        ins=[],
        outs=[],
        sync_info=si,
        bass_nofuse=True,
    )
    nc.register_instruction(inst, overwrite=True)
    return inst


def split_excess_sync(nc, max_waits=1, max_updates=1):
    for f in nc.m.functions:
        for blk in f.blocks:
            insts = blk.instructions
            i = 0
            while i < len(insts):
                inst = insts[i]
                si = inst.sync_info
                if si is None:
                    i += 1
                    continue
                waits = list(si.on_wait or [])
                updates = list(si.on_update or [])
                if len(waits) > max_waits:
                    si.on_wait = waits[:max_waits]
                    extra = waits[max_waits:]
                    new_insts = [
                        _mk_nop(nc, inst.engine, waits=extra[j : j + max_waits])
                        for j in range(0, len(extra), max_waits)
                    ]
                    insts[i:i] = new_insts
                    i += len(new_insts)
                if len(updates) > max_updates:
                    si.on_update = updates[:max_updates]
                    extra = updates[max_updates:]
                    new_insts = [
                        _mk_nop(nc, inst.engine, updates=extra[j : j + max_updates])
                        for j in range(0, len(extra), max_updates)
                    ]
                    insts[i + 1 : i + 1] = new_insts
                    i += len(new_insts)
                i += 1


# ---------------------------------------------------------------------------
def build_nc(H=128):
    NS = H // SH
    HP = H + 2

    nc = bass.Bass("TRN2", target_bir_lowering=False, debug=False, num_devices=8)

    # f32r DRAM views (same bits as f32) let the plain SP/ACT DMA queues
    # feed f32r SBUF tiles without the gpsimd cast path.
    x_d = nc.dram_tensor("x", [C, H, W], F32R, kind="ExternalInput").ap()
    wp1_d = nc.dram_tensor("wp1", [18, 128, 128], F32R, kind="ExternalInput").ap()
    wp2_d = nc.dram_tensor("wp2", [18, 128, 128], F32R, kind="ExternalInput").ap()
    wp_d = nc.dram_tensor("wp", [9, 128, 256], F32R, kind="ExternalInput").ap()
    wc1_d = nc.dram_tensor("wc1", [2, 128, 256], F32R, kind="ExternalInput").ap()
    wc2_d = nc.dram_tensor("wc2", [18, 128, 256], F32R, kind="ExternalInput").ap()
    bp1_d = nc.dram_tensor("bp1", [128, 1], F32, kind="ExternalInput").ap()
    bp2_d = nc.dram_tensor("bp2", [128, 1], F32, kind="ExternalInput").ap()
    bpc1_d = nc.dram_tensor("bpc1", [128, 2], F32, kind="ExternalInput").ap()
    bc2_d = nc.dram_tensor("bc2", [128, 2], F32, kind="ExternalInput").ap()
    pool2_d = nc.dram_tensor("pool2_scratch", [M, H, W], F32R).ap()
    out_d = nc.dram_tensor("out", [C, H, W], F32, kind="ExternalOutput").ap()

    with TileContext(nc) as tc:
        with (
            tc.tile_pool(name="bias", bufs=1) as bias_pool,
            tc.tile_pool(name="p1p", bufs=1) as p1p,
            tc.tile_pool(name="wcd", bufs=1) as wcd,
            tc.tile_pool(name="rring", bufs=1) as rring,
            tc.tile_pool(name="xc", bufs=2) as xcp,
            tc.tile_pool(name="p2l", bufs=2) as p2lp,
            tc.tile_pool(name="psum", bufs=8, space="PSUM") as psum_pool,
        ):
            bp1 = bias_pool.tile([128, 1], F32, name="bp1")
            bp2 = bias_pool.tile([128, 1], F32, name="bp2")
            bpc1 = bias_pool.tile([128, 2], F32, name="bpc1")
            bc2 = bias_pool.tile([128, 2], F32, name="bc2")
            for t, d in ((bp1, bp1_d), (bp2, bp2_d), (bpc1, bpc1_d), (bc2, bc2_d)):
                nc.gpsimd.dma_start(out=t[:, :], in_=d[:, :])

            # phase C/D weights: allocated outside the AB pools so their DMAs
            # (emitted mid-AB, off the startup critical path) run during
            # phase AB instead of stalling at the boundary.
            wp = wcd.tile([128, 9, 256], F32R, name="wpt")
            wc1 = wcd.tile([128, 2, 256], F32R, name="wc1t")
            wc2 = wcd.tile([128, 18, 256], F32R, name="wc2t")

            def load_cd_weights():
                nc.sync.dma_start(out=wp[:, :, :], in_=wp_d.rearrange("t i o -> i t o"))
                nc.sync.dma_start(out=wc1[:, :, :], in_=wc1_d.rearrange("t i o -> i t o"))
                nc.scalar.dma_start(
                    out=wc2[:, 0:9, :], in_=wc2_d[0:9].rearrange("t i o -> i t o")
                )
                nc.sync.dma_start(
                    out=wc2[:, 9:18, :], in_=wc2_d[9:18].rearrange("t i o -> i t o")
                )

            # p1 / pool1 / s image buffer (padded). Interior is fully written
            # by evictions; only the pad ring needs zeroing (read by phase-C
            # taps). Off the critical path -> gpsimd.
            p1buf = p1p.tile([128, HP, W + 2], F32R, name="p1buf")
            nc.gpsimd.memset(p1buf[:, 0:1, :].bitcast(F32), 0.0)
            nc.gpsimd.memset(p1buf[:, HP - 1 : HP, :].bitcast(F32), 0.0)
            nc.gpsimd.memset(p1buf[:, :, 0:1].bitcast(F32), 0.0)
            nc.gpsimd.memset(p1buf[:, :, W + 1 : W + 2].bitcast(F32), 0.0)

            # ---------------- Phase AB: p1 + p2 conv strips, bottom-up -----
            with (
                tc.tile_pool(name="w12", bufs=1) as w12,
                tc.tile_pool(name="xab", bufs=2) as xab,
                tc.tile_pool(name="p2s", bufs=3) as p2sp,
            ):
                wp1 = w12.tile([128, 18, 128], F32R, name="wp1t")
                wp2 = w12.tile([128, 18, 128], F32R, name="wp2t")
                # halves: the first strip's kb0 matmuls only need taps 0-8
                nc.scalar.dma_start(out=wp1[:, 0:9, :], in_=wp1_d[0:9].rearrange("t i o -> i t o"))
                nc.scalar.dma_start(out=wp1[:, 9:18, :], in_=wp1_d[9:18].rearrange("t i o -> i t o"))
                nc.scalar.dma_start(out=wp2[:, 0:9, :], in_=wp2_d[0:9].rearrange("t i o -> i t o"))
                nc.scalar.dma_start(out=wp2[:, 9:18, :], in_=wp2_d[9:18].rearrange("t i o -> i t o"))

                def phase_c_add(s):
                    # s-add slice (disjoint across strips; includes the row
                    # above the strip so p_conv's dy=0 halo row is complete
                    # once this strip's add is done). Emitted inside the AB
                    # loop so DVE runs it well before phase C needs it.
                    h0a = s * SH
                    alo = max(h0a - 1, 0)
                    ahi = h0a + SH - 1 if s < NS - 1 else H
                    p2l = p2lp.tile([128, SH + 1, W], F32R, name="p2l", tag="p2l")
                    nr = ahi - alo
                    nc.scalar.dma_start(
                        out=p2l[:, 0:nr, :], in_=pool2_d[:, alo:ahi, :]
                    )
                    nc.vector.tensor_tensor(
                        out=p1buf[:, 1 + alo : 1 + ahi, 1 : W + 1],
                        in0=p1buf[:, 1 + alo : 1 + ahi, 1 : W + 1],
                        in1=p2l[:, 0:nr, :],
                        op=ADD,
                    )

                for s in range(NS - 1, -1, -1):
                    if s == max(NS - 5, 0):
                        load_cd_weights()
                    h0 = s * SH
                    xt = []
                    for kb in range(2):
                        t = xab.tile(
                            [128, SH + 2, W + 2], F32R, name=f"xab{kb}", tag=f"xab{kb}"
                        )
                        nc.gpsimd.memset(t[:, :, 0:1].bitcast(F32), 0.0)
                        nc.gpsimd.memset(t[:, :, W + 1 : W + 2].bitcast(F32), 0.0)
                        glo = max(h0 - 1, 0)
                        ghi = min(h0 + SH + 1, H)
                        brow = glo - (h0 - 1)
                        if s == NS - 1:
                            # first strip: split so the first PSUM group's rows
                            # land before the whole strip finishes loading
                            gmid = glo + 6
                            nc.sync.dma_start(
                                out=t[:, brow : brow + 6, 1 : W + 1],
                                in_=x_d[kb * 128 : (kb + 1) * 128, glo:gmid, :],
                            )
                            nc.sync.dma_start(
                                out=t[:, brow + 6 : brow + (ghi - glo), 1 : W + 1],
                                in_=x_d[kb * 128 : (kb + 1) * 128, gmid:ghi, :],
                            )
                        else:
                            nc.sync.dma_start(
                                out=t[:, brow : brow + (ghi - glo), 1 : W + 1],
                                in_=x_d[kb * 128 : (kb + 1) * 128, glo:ghi, :],
                            )
                        if s == 0:
                            nc.gpsimd.memset(t[:, 0:1, :].bitcast(F32), 0.0)
                        if s == NS - 1:
                            nc.gpsimd.memset(t[:, SH + 1 : SH + 2, :].bitcast(F32), 0.0)
                        xt.append(t)

                    # p1 conv -> p1buf rows
                    for nt in range(NT):
                        ps = psum_pool.tile([128, 4, W], F32, name="ps1", tag="ps")
                        n = 0
                        for kb in range(2):
                            for dy in range(3):
                                for dx in range(3):
                                    nc.tensor.matmul(
                                        ps[:, :, :],
                                        wp1[:, kb * 9 + dy * 3 + dx, :],
                                        xt[kb][:, nt * 4 + dy : nt * 4 + dy + 4, dx : dx + W],
                                        start=(n == 0),
                                        stop=(n == 17),
                                    )
                                    n += 1
                        gh = h0 + nt * 4
                        nc.scalar.activation(
                            p1buf[:, 1 + gh : 5 + gh, 1 : W + 1],
                            ps[:, :, :],
                            RELU,
                            bias=bp1[:, 0:1],
                        )

                    # p2 conv -> strip tile, in-place W suffix-max, -> DRAM
                    p2t = p2sp.tile([128, SH, W], F32R, name="p2t", tag="p2t")
                    for nt in range(NT):
                        ps = psum_pool.tile([128, 4, W], F32, name="ps2", tag="ps")
                        n = 0
                        for kb in range(2):
                            for dy in range(3):
                                for dx in range(3):
                                    nc.tensor.matmul(
                                        ps[:, :, :],
                                        wp2[:, kb * 9 + dy * 3 + dx, :],
                                        xt[kb][:, nt * 4 + dy : nt * 4 + dy + 4, dx : dx + W],
                                        start=(n == 0),
                                        stop=(n == 17),
                                    )
                                    n += 1
                        nc.scalar.activation(
                            p2t[:, nt * 4 : nt * 4 + 4, :],
                            ps[:, :, :],
                            RELU,
                            bias=bp2[:, 0:1],
                        )
                    d = 1
                    while d < W:
                        nc.vector.tensor_tensor(
                            out=p2t[:, :, 0 : W - d],
                            in0=p2t[:, :, 0 : W - d],
                            in1=p2t[:, :, d:W],
                            op=MAX,
                        )
                        d *= 2
                    nc.sync.dma_start(out=pool2_d[:, h0 : h0 + SH, :], in_=p2t[:, :, :])

                    # pool1 row chain for this strip (row h = max(row h, row h+1),
                    # bottom-up; row h+1 already chained by the earlier strip).
                    for h in range(min(h0 + SH - 1, H - 2), h0 - 1, -1):
                        nc.vector.tensor_tensor(
                            out=p1buf[:, 1 + h : 2 + h, 1 : W + 1],
                            in0=p1buf[:, 1 + h : 2 + h, 1 : W + 1],
                            in1=p1buf[:, 2 + h : 3 + h, 1 : W + 1],
                            op=MAX,
                        )
                    if s + 1 <= NS - 1:
                        phase_c_add(s + 1)
                phase_c_add(0)

            # ---------------- Phase C+D interleaved, bottom-up -------------
            with (
                tc.tile_pool(name="ost", bufs=3) as ost,
            ):
                # r ring: [mb][slot] padded strip buffers
                rslot = [
                    [
                        rring.tile([128, SH + 2, W + 2], F32R, name=f"rs{mb}_{k}")
                        for k in range(3)
                    ]
                    for mb in range(2)
                ]
                for mb in range(2):
                    for k in range(3):
                        nc.gpsimd.memset(rslot[mb][k][:, :, 0:1].bitcast(F32), 0.0)
                        nc.gpsimd.memset(
                            rslot[mb][k][:, :, W + 1 : W + 2].bitcast(F32), 0.0
                        )

                def phase_c(s):
                    h0 = s * SH
                    xc = []
                    for kb in range(2):
                        t = xcp.tile([128, SH, W], F32R, name=f"xc{kb}", tag=f"xc{kb}")
                        nc.scalar.dma_start(
                            out=t[:, :, :],
                            in_=x_d[kb * 128 : (kb + 1) * 128, h0 : h0 + SH, :],
                        )
                        xc.append(t)
                    for mb in range(2):
                        slot = rslot[mb][s % 3]
                        for nt in range(NT):
                            ps = psum_pool.tile([128, 4, W], F32, name="psc", tag="ps")
                            for kb in range(2):
                                nc.tensor.matmul(
                                    ps[:, :, :],
                                    wc1[:, kb, mb * 128 : (mb + 1) * 128],
                                    xc[kb][:, nt * 4 : nt * 4 + 4, :],
                                    start=(kb == 0),
                                    stop=False,
                                )
                            gh = h0 + nt * 4
                            for i in range(9):
                                dy, dx = divmod(i, 3)
                                nc.tensor.matmul(
                                    ps[:, :, :],
                                    wp[:, i, mb * 128 : (mb + 1) * 128],
                                    p1buf[:, gh + dy : gh + dy + 4, dx : dx + W],
                                    start=False,
                                    stop=(i == 8),
                                )
                            nc.scalar.activation(
                                slot[:, 1 + nt * 4 : 5 + nt * 4, 1 : W + 1],
                                ps[:, :, :],
                                RELU,
                                bias=bpc1[:, mb : mb + 1],
                            )

                def phase_d(s):
                    h0 = s * SH
                    for mb in range(2):
                        slot = rslot[mb][s % 3]
                        # halo rows: bottom (global row h0+SH) from strip s+1's
                        # first interior row; top (global row h0-1) from strip
                        # s-1's last interior row.
                        if s == NS - 1:
                            nc.gpsimd.memset(
                                slot[:, SH + 1 : SH + 2, :].bitcast(F32), 0.0
                            )
                        else:
                            nc.scalar.copy(
                                slot[:, SH + 1 : SH + 2, :],
                                rslot[mb][(s + 1) % 3][:, 1:2, :],
                            )
                        if s == 0:
                            nc.gpsimd.memset(slot[:, 0:1, :].bitcast(F32), 0.0)
                        else:
                            nc.scalar.copy(
                                slot[:, 0:1, :], rslot[mb][(s - 1) % 3][:, SH : SH + 1, :]
                            )
                    for mb in range(2):
                        for nt in range(NT):
                            ps = psum_pool.tile([128, 4, W], F32, name="psd", tag="ps")
                            n = 0
                            for kb in range(2):
                                for dy in range(3):
                                    for dx in range(3):
                                        nc.tensor.matmul(
                                            ps[:, :, :],
                                            wc2[:, kb * 9 + dy * 3 + dx, mb * 128 : (mb + 1) * 128],
                                            rslot[kb][s % 3][
                                                :, nt * 4 + dy : nt * 4 + dy + 4, dx : dx + W
                                            ],
                                            start=(n == 0),
                                            stop=(n == 17),
                                        )
                                        n += 1
                            ot = ost.tile([128, 4, W], F32, name="otile", tag="otile")
                            nc.scalar.activation(
                                ot[:, :, :], ps[:, :, :], RELU, bias=bc2[:, mb : mb + 1]
                            )
                            gh = h0 + nt * 4
                            nc.sync.dma_start(
                                out=out_d[mb * 128 : (mb + 1) * 128, gh : gh + 4, :],
                                in_=ot[:, :, :],
                            )

                phase_c(NS - 1)
                for s in range(NS - 2, -1, -1):
                    phase_c(s)
                    phase_d(s + 1)
                phase_d(0)

    split_excess_sync(nc)
    return nc


# ---------------------------------------------------------------------------
def _fold(Wc, g, b, m, v):
    scale = (g / np.sqrt(v + EPS)).astype(np.float64)
    Wf = Wc.astype(np.float64) * scale[:, None, None, None]
    bias = b.astype(np.float64) - m.astype(np.float64) * scale
    return Wf.astype(np.float32), bias.astype(np.float32)


def _pack3x3(Wf):
    O, I = Wf.shape[:2]
    n_kb = I // 128
    out = np.empty((n_kb * 9, 128, O), dtype=np.float32)
    for kb in range(n_kb):
        for dy in range(3):
            for dx in range(3):
                out[kb * 9 + dy * 3 + dx] = Wf[:, kb * 128 : (kb + 1) * 128, dy, dx].T
    return out


def _prep_weights(inp):
    wp1f, bp1 = _fold(inp["W_p1"], inp["g_p1"], inp["b_p1"], inp["m_p1"], inp["v_p1"])
    wp2f, bp2 = _fold(inp["W_p2"], inp["g_p2"], inp["b_p2"], inp["m_p2"], inp["v_p2"])
    wpf, bp = _fold(inp["W_p"], inp["g_p"], inp["b_p"], inp["m_p"], inp["v_p"])
    wc1f, bc1 = _fold(inp["W_c1"], inp["g_c1"], inp["b_c1"], inp["m_c1"], inp["v_c1"])
    wc2f, bc2 = _fold(inp["W_c2"], inp["g_c2"], inp["b_c2"], inp["m_c2"], inp["v_c2"])
    return {
        "wp1": _pack3x3(wp1f),
        "wp2": _pack3x3(wp2f),
        "wp": _pack3x3(wpf),
        "wc2": _pack3x3(wc2f),
        "wc1": np.stack(
            [wc1f[:, kb * 128 : (kb + 1) * 128, 0, 0].T for kb in range(2)]
        ).astype(np.float32),
        "bp1": bp1.reshape(128, 1),
        "bp2": bp2.reshape(128, 1),
        "bpc1": (bp + bc1).reshape(2, 128).T.copy(),
        "bc2": bc2.reshape(2, 128).T.copy(),
    }


_nc_cache = {}


def _get_nc(H):
    if H not in _nc_cache:
        _nc_cache[H] = build_nc(H)
    return _nc_cache[H]


def run(inputs, H=128, trace=False):
    nc = _get_nc(H)
    inputs = {k: np.asarray(v) for k, v in inputs.items()}
    wd = _prep_weights(inputs)
    x = np.asarray(inputs["x"], dtype=np.float32)
    B = x.shape[0]
    in_maps = [dict(wd, x=np.ascontiguousarray(x[i, :, :H, :])) for i in range(B)]
    res = run_bass_kernel_spmd(nc, in_maps, core_ids=list(range(B)), trace=trace)
    out = np.stack([res.results[i]["out"] for i in range(B)])
    return out, res


def kernel(**inputs):
    out, _ = run(inputs, H=128, trace=False)
    return out



# revision 2
# speedup vs baseline: 1.0911x; 1.0911x over previous
"""Trainium2 Bass kernel for nn_BDPool (corner-pool style block).

Per-sample network (NCHW, x: (256,128,128)):
    p1 = relu(bn1(conv3x3_256to128(x)))
    p2 = relu(bn2(conv3x3_256to128(x)))
    pool1 = reverse-cummax_H(p1); pool2 = reverse-cummax_W(p2)
    r  = relu(bn_p(conv3x3_128to256(pool1+pool2)) + bn_c1(conv1x1_256to256(x)))
    out = relu(bn_c2(conv3x3_256to256(r)))

Sharding: data-parallel over batch; core i computes sample i entirely.

Implementation notes:
- All conv operands (weights + activations) are bf16; PSUM accumulation is
  fp32, biases fp32. Inputs are cast to bf16 host-side. This halves
  LDWEIGHTS bytes (f32 LDW ~188ns barely hides under a 213ns matmul) and
  halves DMA + vector-engine element traffic.
- BN folded into conv weights (scale) + per-channel bias applied during
  PSUM->SBUF eviction on the scalar engine (Relu activation with bias AP).
- 3x3 convs are 9 shifted bf16 matmuls accumulating in PSUM; inputs live
  in SBUF with 1-pixel zero-padded borders (row stride W+2).
- pool2 (reverse cummax along W) is a 7-step in-place log-shift max per
  strip, overlapped inside the conv phase; strips round-trip via DRAM.
- pool1 (reverse cummax along H) is a 127-step row max-chain emitted
  bottom-up interleaved with the conv strips so it overlaps on DVE.
- Phases are pipelined bottom-up: p1/p2 conv strips -> per-strip s-add ->
  p_conv+conv1 (fused PSUM group) -> final conv, with the intermediate r
  held in a 3-deep SBUF ring (no DRAM round trip).
"""

import numpy as np
import ml_dtypes

import concourse.bass as bass
import concourse.mybir as mybir
from concourse.tile import TileContext
from concourse.bass_utils import run_bass_kernel_spmd

dt = mybir.dt
F32 = dt.float32
BF16 = dt.bfloat16
RELU = mybir.ActivationFunctionType.Relu
MAX = mybir.AluOpType.max
ADD = mybir.AluOpType.add

C = 256
M = 128
W = 128
SH = 8  # strip height
NT = SH // 4

EPS = 1e-5

NP_BF16 = ml_dtypes.bfloat16


# ---------------------------------------------------------------------------
# walrus wait-limit workaround: split instructions carrying >1 sem wait (or
# >1 sem update) into a chain of NOPs each carrying one.
_wfix_counter = [0]


def _mk_nop(nc, engine, waits=None, updates=None):
    _wfix_counter[0] += 1
    si = mybir.SyncInfo(on_wait=list(waits or []), on_update=list(updates or []))
    inst = mybir.InstNoOp(
        name=f"WFIX-{_wfix_counter[0]}",
        engine=engine,
        ins=[],
        outs=[],
        sync_info=si,
        bass_nofuse=True,
    )
    nc.register_instruction(inst, overwrite=True)
    return inst


def split_excess_sync(nc, max_waits=1, max_updates=1):
    for f in nc.m.functions:
        for blk in f.blocks:
            insts = blk.instructions
            i = 0
            while i < len(insts):
                inst = insts[i]
                si = inst.sync_info
                if si is None:
                    i += 1
                    continue
                waits = list(si.on_wait or [])
                updates = list(si.on_update or [])
                if len(waits) > max_waits:
                    si.on_wait = waits[:max_waits]
                    extra = waits[max_waits:]
                    new_insts = [
                        _mk_nop(nc, inst.engine, waits=extra[j : j + max_waits])
                        for j in range(0, len(extra), max_waits)
                    ]
                    insts[i:i] = new_insts
                    i += len(new_insts)
                if len(updates) > max_updates:
                    si.on_update = updates[:max_updates]
                    extra = updates[max_updates:]
                    new_insts = [
                        _mk_nop(nc, inst.engine, updates=extra[j : j + max_updates])
                        for j in range(0, len(extra), max_updates)
                    ]
                    insts[i + 1 : i + 1] = new_insts
                    i += len(new_insts)
                i += 1


# ---------------------------------------------------------------------------
def build_nc(H=128):
    NS = H // SH
    HP = H + 2

    nc = bass.Bass("TRN2", target_bir_lowering=False, debug=False, num_devices=8)

    x_d = nc.dram_tensor("x", [C, H, W], BF16, kind="ExternalInput").ap()
    wp1_d = nc.dram_tensor("wp1", [18, 128, 128], BF16, kind="ExternalInput").ap()
    wp2_d = nc.dram_tensor("wp2", [18, 128, 128], BF16, kind="ExternalInput").ap()
    wp_d = nc.dram_tensor("wp", [9, 128, 256], BF16, kind="ExternalInput").ap()
    wc1_d = nc.dram_tensor("wc1", [2, 128, 256], BF16, kind="ExternalInput").ap()
    wc2_d = nc.dram_tensor("wc2", [18, 128, 256], BF16, kind="ExternalInput").ap()
    bp1_d = nc.dram_tensor("bp1", [128, 1], F32, kind="ExternalInput").ap()
    bp2_d = nc.dram_tensor("bp2", [128, 1], F32, kind="ExternalInput").ap()
    bpc1_d = nc.dram_tensor("bpc1", [128, 2], F32, kind="ExternalInput").ap()
    bc2_d = nc.dram_tensor("bc2", [128, 2], F32, kind="ExternalInput").ap()
    pool2_d = nc.dram_tensor("pool2_scratch", [M, H, W], BF16).ap()
    out_d = nc.dram_tensor("out", [C, H, W], F32, kind="ExternalOutput").ap()

    with TileContext(nc) as tc:
        with (
            tc.tile_pool(name="bias", bufs=1) as bias_pool,
            tc.tile_pool(name="p1p", bufs=1) as p1p,
            tc.tile_pool(name="wcd", bufs=1) as wcd,
            tc.tile_pool(name="rring", bufs=1) as rring,
            tc.tile_pool(name="xc", bufs=2) as xcp,
            tc.tile_pool(name="p2l", bufs=2) as p2lp,
            tc.tile_pool(name="psum", bufs=8, space="PSUM") as psum_pool,
        ):
            bp1 = bias_pool.tile([128, 1], F32, name="bp1")
            bp2 = bias_pool.tile([128, 1], F32, name="bp2")
            bpc1 = bias_pool.tile([128, 2], F32, name="bpc1")
            bc2 = bias_pool.tile([128, 2], F32, name="bc2")
            for t, d in ((bp1, bp1_d), (bp2, bp2_d), (bpc1, bpc1_d), (bc2, bc2_d)):
                nc.gpsimd.dma_start(out=t[:, :], in_=d[:, :])

            # phase C/D weights: allocated outside the AB pools so their DMAs
            # (emitted mid-AB, off the startup critical path) run during
            # phase AB instead of stalling at the boundary.
            wp = wcd.tile([128, 9, 256], BF16, name="wpt")
            wc1 = wcd.tile([128, 2, 256], BF16, name="wc1t")
            wc2 = wcd.tile([128, 18, 256], BF16, name="wc2t")

            def load_cd_weights():
                nc.sync.dma_start(out=wp[:, :, :], in_=wp_d.rearrange("t i o -> i t o"))
                nc.sync.dma_start(out=wc1[:, :, :], in_=wc1_d.rearrange("t i o -> i t o"))
                nc.scalar.dma_start(
                    out=wc2[:, 0:9, :], in_=wc2_d[0:9].rearrange("t i o -> i t o")
                )
                nc.sync.dma_start(
                    out=wc2[:, 9:18, :], in_=wc2_d[9:18].rearrange("t i o -> i t o")
                )

            # p1 / pool1 / s image buffer (padded). Interior is fully written
            # by evictions; only the pad ring needs zeroing (read by phase-C
            # taps). Off the critical path -> gpsimd.
            p1buf = p1p.tile([128, HP, W + 2], BF16, name="p1buf")
            nc.gpsimd.memset(p1buf[:, 0:1, :], 0.0)
            nc.gpsimd.memset(p1buf[:, HP - 1 : HP, :], 0.0)
            nc.gpsimd.memset(p1buf[:, :, 0:1], 0.0)
            nc.gpsimd.memset(p1buf[:, :, W + 1 : W + 2], 0.0)

            # ---------------- Phase AB: p1 + p2 conv strips, bottom-up -----
            with (
                tc.tile_pool(name="w12", bufs=1) as w12,
                tc.tile_pool(name="xab", bufs=2) as xab,
                tc.tile_pool(name="p2s", bufs=3) as p2sp,
            ):
                wp1 = w12.tile([128, 18, 128], BF16, name="wp1t")
                wp2 = w12.tile([128, 18, 128], BF16, name="wp2t")
                # halves: the first strip's kb0 matmuls only need taps 0-8
                nc.scalar.dma_start(out=wp1[:, 0:9, :], in_=wp1_d[0:9].rearrange("t i o -> i t o"))
                nc.scalar.dma_start(out=wp1[:, 9:18, :], in_=wp1_d[9:18].rearrange("t i o -> i t o"))
                nc.scalar.dma_start(out=wp2[:, 0:9, :], in_=wp2_d[0:9].rearrange("t i o -> i t o"))
                nc.scalar.dma_start(out=wp2[:, 9:18, :], in_=wp2_d[9:18].rearrange("t i o -> i t o"))

                def phase_c_add(s):
                    # s-add slice (disjoint across strips; includes the row
                    # above the strip so p_conv's dy=0 halo row is complete
                    # once this strip's add is done). Emitted inside the AB
                    # loop so DVE runs it well before phase C needs it.
                    h0a = s * SH
                    alo = max(h0a - 1, 0)
                    ahi = h0a + SH - 1 if s < NS - 1 else H
                    p2l = p2lp.tile([128, SH + 1, W], BF16, name="p2l", tag="p2l")
                    nr = ahi - alo
                    nc.scalar.dma_start(
                        out=p2l[:, 0:nr, :], in_=pool2_d[:, alo:ahi, :]
                    )
                    nc.vector.tensor_tensor(
                        out=p1buf[:, 1 + alo : 1 + ahi, 1 : W + 1],
                        in0=p1buf[:, 1 + alo : 1 + ahi, 1 : W + 1],
                        in1=p2l[:, 0:nr, :],
                        op=ADD,
                    )

                for s in range(NS - 1, -1, -1):
                    if s == max(NS - 5, 0):
                        load_cd_weights()
                    h0 = s * SH
                    xt = []
                    for kb in range(2):
                        t = xab.tile(
                            [128, SH + 2, W + 2], BF16, name=f"xab{kb}", tag=f"xab{kb}"
                        )
                        nc.gpsimd.memset(t[:, :, 0:1], 0.0)
                        nc.gpsimd.memset(t[:, :, W + 1 : W + 2], 0.0)
                        glo = max(h0 - 1, 0)
                        ghi = min(h0 + SH + 1, H)
                        brow = glo - (h0 - 1)
                        if s == NS - 1:
                            # first strip: split so the first PSUM group's rows
                            # land before the whole strip finishes loading
                            gmid = glo + 6
                            nc.sync.dma_start(
                                out=t[:, brow : brow + 6, 1 : W + 1],
                                in_=x_d[kb * 128 : (kb + 1) * 128, glo:gmid, :],
                            )
                            nc.sync.dma_start(
                                out=t[:, brow + 6 : brow + (ghi - glo), 1 : W + 1],
                                in_=x_d[kb * 128 : (kb + 1) * 128, gmid:ghi, :],
                            )
                        else:
                            nc.sync.dma_start(
                                out=t[:, brow : brow + (ghi - glo), 1 : W + 1],
                                in_=x_d[kb * 128 : (kb + 1) * 128, glo:ghi, :],
                            )
                        if s == 0:
                            nc.gpsimd.memset(t[:, 0:1, :], 0.0)
                        if s == NS - 1:
                            nc.gpsimd.memset(t[:, SH + 1 : SH + 2, :], 0.0)
                        xt.append(t)

                    # p1 conv -> p1buf rows
                    for nt in range(NT):
                        ps = psum_pool.tile([128, 4, W], F32, name="ps1", tag="ps")
                        n = 0
                        for kb in range(2):
                            for dy in range(3):
                                for dx in range(3):
                                    nc.tensor.matmul(
                                        ps[:, :, :],
                                        wp1[:, kb * 9 + dy * 3 + dx, :],
                                        xt[kb][:, nt * 4 + dy : nt * 4 + dy + 4, dx : dx + W],
                                        start=(n == 0),
                                        stop=(n == 17),
                                    )
                                    n += 1
                        gh = h0 + nt * 4
                        nc.scalar.activation(
                            p1buf[:, 1 + gh : 5 + gh, 1 : W + 1],
                            ps[:, :, :],
                            RELU,
                            bias=bp1[:, 0:1],
                        )

                    # p2 conv -> strip tile, in-place W suffix-max, -> DRAM
                    p2t = p2sp.tile([128, SH, W], BF16, name="p2t", tag="p2t")
                    for nt in range(NT):
                        ps = psum_pool.tile([128, 4, W], F32, name="ps2", tag="ps")
                        n = 0
                        for kb in range(2):
                            for dy in range(3):
                                for dx in range(3):
                                    nc.tensor.matmul(
                                        ps[:, :, :],
                                        wp2[:, kb * 9 + dy * 3 + dx, :],
                                        xt[kb][:, nt * 4 + dy : nt * 4 + dy + 4, dx : dx + W],
                                        start=(n == 0),
                                        stop=(n == 17),
                                    )
                                    n += 1
                        nc.scalar.activation(
                            p2t[:, nt * 4 : nt * 4 + 4, :],
                            ps[:, :, :],
                            RELU,
                            bias=bp2[:, 0:1],
                        )
                    d = 1
                    while d < W:
                        nc.vector.tensor_tensor(
                            out=p2t[:, :, 0 : W - d],
                            in0=p2t[:, :, 0 : W - d],
                            in1=p2t[:, :, d:W],
                            op=MAX,
                        )
                        d *= 2
                    nc.sync.dma_start(out=pool2_d[:, h0 : h0 + SH, :], in_=p2t[:, :, :])

                    # pool1 row chain for this strip (row h = max(row h, row h+1),
                    # bottom-up; row h+1 already chained by the earlier strip).
                    for h in range(min(h0 + SH - 1, H - 2), h0 - 1, -1):
                        nc.vector.tensor_tensor(
                            out=p1buf[:, 1 + h : 2 + h, 1 : W + 1],
                            in0=p1buf[:, 1 + h : 2 + h, 1 : W + 1],
                            in1=p1buf[:, 2 + h : 3 + h, 1 : W + 1],
                            op=MAX,
                        )
                    if s + 1 <= NS - 1:
                        phase_c_add(s + 1)
                phase_c_add(0)

            # ---------------- Phase C+D interleaved, bottom-up -------------
            with (
                tc.tile_pool(name="ost", bufs=3) as ost,
            ):
                # r ring: [mb][slot] padded strip buffers
                rslot = [
                    [
                        rring.tile([128, SH + 2, W + 2], BF16, name=f"rs{mb}_{k}")
                        for k in range(3)
                    ]
                    for mb in range(2)
                ]
                for mb in range(2):
                    for k in range(3):
                        nc.gpsimd.memset(rslot[mb][k][:, :, 0:1], 0.0)
                        nc.gpsimd.memset(
                            rslot[mb][k][:, :, W + 1 : W + 2], 0.0
                        )

                def phase_c(s):
                    h0 = s * SH
                    xc = []
                    for kb in range(2):
                        t = xcp.tile([128, SH, W], BF16, name=f"xc{kb}", tag=f"xc{kb}")
                        nc.scalar.dma_start(
                            out=t[:, :, :],
                            in_=x_d[kb * 128 : (kb + 1) * 128, h0 : h0 + SH, :],
                        )
                        xc.append(t)
                    for mb in range(2):
                        slot = rslot[mb][s % 3]
                        for nt in range(NT):
                            ps = psum_pool.tile([128, 4, W], F32, name="psc", tag="ps")
                            for kb in range(2):
                                nc.tensor.matmul(
                                    ps[:, :, :],
                                    wc1[:, kb, mb * 128 : (mb + 1) * 128],
                                    xc[kb][:, nt * 4 : nt * 4 + 4, :],
                                    start=(kb == 0),
                                    stop=False,
                                )
                            gh = h0 + nt * 4
                            for i in range(9):
                                dy, dx = divmod(i, 3)
                                nc.tensor.matmul(
                                    ps[:, :, :],
                                    wp[:, i, mb * 128 : (mb + 1) * 128],
                                    p1buf[:, gh + dy : gh + dy + 4, dx : dx + W],
                                    start=False,
                                    stop=(i == 8),
                                )
                            nc.scalar.activation(
                                slot[:, 1 + nt * 4 : 5 + nt * 4, 1 : W + 1],
                                ps[:, :, :],
                                RELU,
                                bias=bpc1[:, mb : mb + 1],
                            )

                def phase_d(s):
                    h0 = s * SH
                    for mb in range(2):
                        slot = rslot[mb][s % 3]
                        # halo rows: bottom (global row h0+SH) from strip s+1's
                        # first interior row; top (global row h0-1) from strip
                        # s-1's last interior row.
                        if s == NS - 1:
                            nc.gpsimd.memset(
                                slot[:, SH + 1 : SH + 2, :], 0.0
                            )
                        else:
                            nc.scalar.copy(
                                slot[:, SH + 1 : SH + 2, :],
                                rslot[mb][(s + 1) % 3][:, 1:2, :],
                            )
                        if s == 0:
                            nc.gpsimd.memset(slot[:, 0:1, :], 0.0)
                        else:
                            nc.scalar.copy(
                                slot[:, 0:1, :], rslot[mb][(s - 1) % 3][:, SH : SH + 1, :]
                            )
                    for mb in range(2):
                        for nt in range(NT):
                            ps = psum_pool.tile([128, 4, W], F32, name="psd", tag="ps")
                            n = 0
                            for kb in range(2):
                                for dy in range(3):
                                    for dx in range(3):
                                        nc.tensor.matmul(
                                            ps[:, :, :],
                                            wc2[:, kb * 9 + dy * 3 + dx, mb * 128 : (mb + 1) * 128],
                                            rslot[kb][s % 3][
                                                :, nt * 4 + dy : nt * 4 + dy + 4, dx : dx + W
                                            ],
                                            start=(n == 0),
                                            stop=(n == 17),
                                        )
                                        n += 1
                            ot = ost.tile([128, 4, W], F32, name="otile", tag="otile")
                            nc.scalar.activation(
                                ot[:, :, :], ps[:, :, :], RELU, bias=bc2[:, mb : mb + 1]
                            )
                            gh = h0 + nt * 4
                            nc.sync.dma_start(
                                out=out_d[mb * 128 : (mb + 1) * 128, gh : gh + 4, :],
                                in_=ot[:, :, :],
                            )

                phase_c(NS - 1)
                for s in range(NS - 2, -1, -1):
                    phase_c(s)
                    phase_d(s + 1)
                phase_d(0)

    split_excess_sync(nc)
    return nc


# ---------------------------------------------------------------------------
def _fold(Wc, g, b, m, v):
    scale = (g / np.sqrt(v + EPS)).astype(np.float64)
    Wf = Wc.astype(np.float64) * scale[:, None, None, None]
    bias = b.astype(np.float64) - m.astype(np.float64) * scale
    return Wf.astype(np.float32), bias.astype(np.float32)


def _pack3x3(Wf):
    O, I = Wf.shape[:2]
    n_kb = I // 128
    out = np.empty((n_kb * 9, 128, O), dtype=NP_BF16)
    for kb in range(n_kb):
        for dy in range(3):
            for dx in range(3):
                out[kb * 9 + dy * 3 + dx] = Wf[:, kb * 128 : (kb + 1) * 128, dy, dx].T.astype(NP_BF16)
    return out


def _prep_weights(inp):
    wp1f, bp1 = _fold(inp["W_p1"], inp["g_p1"], inp["b_p1"], inp["m_p1"], inp["v_p1"])
    wp2f, bp2 = _fold(inp["W_p2"], inp["g_p2"], inp["b_p2"], inp["m_p2"], inp["v_p2"])
    wpf, bp = _fold(inp["W_p"], inp["g_p"], inp["b_p"], inp["m_p"], inp["v_p"])
    wc1f, bc1 = _fold(inp["W_c1"], inp["g_c1"], inp["b_c1"], inp["m_c1"], inp["v_c1"])
    wc2f, bc2 = _fold(inp["W_c2"], inp["g_c2"], inp["b_c2"], inp["m_c2"], inp["v_c2"])
    return {
        "wp1": _pack3x3(wp1f),
        "wp2": _pack3x3(wp2f),
        "wp": _pack3x3(wpf),
        "wc2": _pack3x3(wc2f),
        "wc1": np.stack(
            [wc1f[:, kb * 128 : (kb + 1) * 128, 0, 0].T for kb in range(2)]
        ).astype(NP_BF16),
        "bp1": bp1.reshape(128, 1),
        "bp2": bp2.reshape(128, 1),
        "bpc1": (bp + bc1).reshape(2, 128).T.copy(),
        "bc2": bc2.reshape(2, 128).T.copy(),
    }


_nc_cache = {}


def _get_nc(H):
    if H not in _nc_cache:
        _nc_cache[H] = build_nc(H)
    return _nc_cache[H]


def run(inputs, H=128, trace=False):
    nc = _get_nc(H)
    inputs = {k: np.asarray(v) for k, v in inputs.items()}
    wd = _prep_weights(inputs)
    x = np.asarray(inputs["x"], dtype=np.float32).astype(NP_BF16)
    B = x.shape[0]
    in_maps = [dict(wd, x=np.ascontiguousarray(x[i, :, :H, :])) for i in range(B)]
    res = run_bass_kernel_spmd(nc, in_maps, core_ids=list(range(B)), trace=trace)
    out = np.stack([res.results[i]["out"] for i in range(B)])
    return out, res


def kernel(**inputs):
    out, _ = run(inputs, H=128, trace=False)
    return out


# revision 14
# speedup vs baseline: 1.4015x; 1.2846x over previous
"""Trainium2 Bass kernel for nn_BDPool (corner-pool style block).

Per-sample network (NCHW, x: (256,128,128)):
    p1 = relu(bn1(conv3x3_256to128(x)))
    p2 = relu(bn2(conv3x3_256to128(x)))
    pool1 = reverse-cummax_H(p1); pool2 = reverse-cummax_W(p2)
    r  = relu(bn_p(conv3x3_128to256(pool1+pool2)) + bn_c1(conv1x1_256to256(x)))
    out = relu(bn_c2(conv3x3_256to256(r)))

Sharding: data-parallel over batch; core i computes sample i entirely.

Implementation notes:
- All conv operands are bf16 (inputs cast host-side); PSUM + A^T combine
  temps are fp32, biases fp32.
- 3x3 convs use 1-D Winograd F(2,3) along H (1.5x fewer moving columns):
  per row-pair tile, 4 transformed row-planes d0..d3 are built on gpsimd
  (d0=X0-X2, d1=X1+X2, d2=X2-X1, d3=X1-X3), matmuls per (m, kb, dx)
  accumulate m-planes in 4 PSUM banks, and the A^T combine
  (y0=m0+m1+m2, y1=m1-m2-m3) runs on DVE into f32 temps, evicted with
  relu+bias on the scalar engine into stride-2 row slices.
- Winograd weight transform (G w, incl. BN fold) is done host-side in f64.
- conv1x1 (c1) is fused into phase C's PSUM groups: +c1(even rows) into
  m0, -c1(odd rows) into m3 (negated weights), so y0/y1 pick it up with
  the right sign.
- pool2 (reverse cummax along W) is a 7-step in-place log-shift max per
  strip; strips round-trip via DRAM. pool1 (reverse cummax along H) is a
  127-step row max-chain emitted bottom-up interleaved with the strips.
- Phases are pipelined bottom-up as before; r strips live in a 3-deep
  SBUF ring.
"""

import numpy as np
import ml_dtypes

import concourse.bass as bass
import concourse.mybir as mybir
from concourse.tile import TileContext
from concourse.bass_utils import run_bass_kernel_spmd

dt = mybir.dt
F32 = dt.float32
BF16 = dt.bfloat16
RELU = mybir.ActivationFunctionType.Relu
MAX = mybir.AluOpType.max
ADD = mybir.AluOpType.add
SUB = mybir.AluOpType.subtract

C = 256
M = 128
W = 128
SH = 8  # strip height (4 winograd row tiles)

EPS = 1e-5

NP_BF16 = ml_dtypes.bfloat16


# ---------------------------------------------------------------------------
# walrus wait-limit workaround: split instructions carrying >1 sem wait (or
# >1 sem update) into a chain of NOPs each carrying one.
_wfix_counter = [0]


def _mk_nop(nc, engine, waits=None, updates=None):
    _wfix_counter[0] += 1
    si = mybir.SyncInfo(on_wait=list(waits or []), on_update=list(updates or []))
    inst = mybir.InstNoOp(
        name=f"WFIX-{_wfix_counter[0]}",
        engine=engine,
        ins=[],
        outs=[],
        sync_info=si,
        bass_nofuse=True,
    )
    nc.register_instruction(inst, overwrite=True)
    return inst


def split_excess_sync(nc, max_waits=1, max_updates=1):
    for f in nc.m.functions:
        for blk in f.blocks:
            insts = blk.instructions
            i = 0
            while i < len(insts):
                inst = insts[i]
                si = inst.sync_info
                if si is None:
                    i += 1
                    continue
                waits = list(si.on_wait or [])
                updates = list(si.on_update or [])
                if len(waits) > max_waits:
                    si.on_wait = waits[:max_waits]
                    extra = waits[max_waits:]
                    new_insts = [
                        _mk_nop(nc, inst.engine, waits=extra[j : j + max_waits])
                        for j in range(0, len(extra), max_waits)
                    ]
                    insts[i:i] = new_insts
                    i += len(new_insts)
                if len(updates) > max_updates:
                    si.on_update = updates[:max_updates]
                    extra = updates[max_updates:]
                    new_insts = [
                        _mk_nop(nc, inst.engine, updates=extra[j : j + max_updates])
                        for j in range(0, len(extra), max_updates)
                    ]
                    insts[i + 1 : i + 1] = new_insts
                    i += len(new_insts)
                i += 1


# ---------------------------------------------------------------------------
def build_nc(H=128):
    NS = H // SH
    HP = H + 2

    nc = bass.Bass("TRN2", target_bir_lowering=False, debug=False, num_devices=8)

    x_d = nc.dram_tensor("x", [C, H, W], BF16, kind="ExternalInput").ap()
    # winograd-packed 3x3 weights: [kb*12 + dx*4 + m, 128, O]
    wp1_d = nc.dram_tensor("wp1", [24, 128, 128], BF16, kind="ExternalInput").ap()
    wp2_d = nc.dram_tensor("wp2", [24, 128, 128], BF16, kind="ExternalInput").ap()
    wp_d = nc.dram_tensor("wp", [12, 128, 256], BF16, kind="ExternalInput").ap()
    # c1 1x1 weights: [kb0+, kb1+, kb0-, kb1-]
    wc1_d = nc.dram_tensor("wc1", [4, 128, 256], BF16, kind="ExternalInput").ap()
    wc2_d = nc.dram_tensor("wc2", [24, 128, 256], BF16, kind="ExternalInput").ap()
    bp1_d = nc.dram_tensor("bp1", [128, 1], F32, kind="ExternalInput").ap()
    bp2_d = nc.dram_tensor("bp2", [128, 1], F32, kind="ExternalInput").ap()
    bpc1_d = nc.dram_tensor("bpc1", [128, 2], F32, kind="ExternalInput").ap()
    bc2_d = nc.dram_tensor("bc2", [128, 2], F32, kind="ExternalInput").ap()
    pool2_d = nc.dram_tensor("pool2_scratch", [M, H, W], BF16).ap()
    out_d = nc.dram_tensor("out", [C, H, W], F32, kind="ExternalOutput").ap()

    with TileContext(nc) as tc:
        with (
            tc.tile_pool(name="bias", bufs=1) as bias_pool,
            tc.tile_pool(name="p1p", bufs=1) as p1p,
            tc.tile_pool(name="wcd", bufs=1) as wcd,
            tc.tile_pool(name="rring", bufs=1) as rring,
            tc.tile_pool(name="xc", bufs=2) as xcp,
            tc.tile_pool(name="p2l", bufs=2) as p2lp,
            tc.tile_pool(name="ytmp", bufs=2) as ytp,
            tc.tile_pool(name="swp", bufs=2) as swp,
            tc.tile_pool(name="rwp", bufs=2) as rwp,
            tc.tile_pool(name="psum", bufs=8, space="PSUM") as psum_pool,
        ):
            bp1 = bias_pool.tile([128, 1], F32, name="bp1")
            bp2 = bias_pool.tile([128, 1], F32, name="bp2")
            bpc1 = bias_pool.tile([128, 2], F32, name="bpc1")
            bc2 = bias_pool.tile([128, 2], F32, name="bc2")
            for t, d in ((bp1, bp1_d), (bp2, bp2_d), (bpc1, bpc1_d), (bc2, bc2_d)):
                nc.gpsimd.dma_start(out=t[:, :], in_=d[:, :])

            # phase C/D weights: DMAs emitted mid-AB so they run during AB.
            wpt = wcd.tile([128, 12, 256], BF16, name="wpt")
            wc1t = wcd.tile([128, 4, 256], BF16, name="wc1t")
            wc2t = wcd.tile([128, 24, 256], BF16, name="wc2t")

            def load_cd_weights():
                nc.sync.dma_start(out=wpt[:, :, :], in_=wp_d.rearrange("t i o -> i t o"))
                nc.sync.dma_start(out=wc1t[:, :, :], in_=wc1_d.rearrange("t i o -> i t o"))
                nc.scalar.dma_start(
                    out=wc2t[:, 0:12, :], in_=wc2_d[0:12].rearrange("t i o -> i t o")
                )
                nc.sync.dma_start(
                    out=wc2t[:, 12:24, :], in_=wc2_d[12:24].rearrange("t i o -> i t o")
                )

            # p1 / pool1 / s image buffer (padded).
            p1buf = p1p.tile([128, HP, W + 2], BF16, name="p1buf")
            nc.gpsimd.memset(p1buf[:, 0:1, :], 0.0)
            nc.gpsimd.memset(p1buf[:, HP - 1 : HP, :], 0.0)
            nc.gpsimd.memset(p1buf[:, :, 0:1], 0.0)
            nc.gpsimd.memset(p1buf[:, :, W + 1 : W + 2], 0.0)

            def transform(dst, src):
                # dst: [128, 4, 4, W+2] m-planes; src: padded rows [128, 10, W+2]
                # tile j: X0=src[2j], X1=src[2j+1], X2=src[2j+2], X3=src[2j+3]
                # m0 and m3 come from one contiguous difference plane
                # T0[i] = src[i]-src[i+2] (even rows -> m0, odd -> m3), written
                # through a transposed AP into the m-plane layout.
                t0_out = dst[:, 0::3, :, :].transpose([0, 2, 1, 3])
                in0 = src[:, 0:8, :].rearrange("p (j t) c -> p j t c", t=2)
                in1 = src[:, 2:10, :].rearrange("p (j t) c -> p j t c", t=2)
                nc.vector.tensor_tensor(out=t0_out, in0=in0, in1=in1, op=SUB)
                X1 = src[:, 1:9:2, :]
                X2 = src[:, 2:10:2, :]
                nc.vector.tensor_tensor(out=dst[:, 1, :, :], in0=X1, in1=X2, op=ADD)
                nc.vector.tensor_tensor(out=dst[:, 2, :, :], in0=X2, in1=X1, op=SUB)

            def combine_evict(ps, dst_even, dst_odd, bias):
                # y0 = m0+m1+m2, y1 = m1-m2-m3; relu+bias on eviction.
                # The scalar engine evicts each m-plane PSUM->SBUF (bf16), so
                # DVE combines run in the cheap same-dtype bf16 SBUF 2x mode
                # and each PSUM bank has exactly one fast reader.
                sm = []
                for i in range(4):
                    t = ytp.tile([128, SH // 2, W], BF16, name=f"sm{i}", tag=f"sm{i}")
                    nc.scalar.copy(t[:, :, :], ps[i][:, :, :])
                    sm.append(t)
                y0 = ytp.tile([128, SH // 2, W], BF16, name="yt0", tag="yt0")
                y1 = ytp.tile([128, SH // 2, W], BF16, name="yt1", tag="yt1")
                nc.vector.tensor_tensor(out=y0[:, :, :], in0=sm[0][:, :, :], in1=sm[1][:, :, :], op=ADD)
                nc.vector.tensor_tensor(out=y0[:, :, :], in0=y0[:, :, :], in1=sm[2][:, :, :], op=ADD)
                nc.vector.tensor_tensor(out=y1[:, :, :], in0=sm[1][:, :, :], in1=sm[2][:, :, :], op=SUB)
                nc.vector.tensor_tensor(out=y1[:, :, :], in0=y1[:, :, :], in1=sm[3][:, :, :], op=SUB)
                nc.scalar.activation(dst_even, y0[:, :, :], RELU, bias=bias)
                nc.scalar.activation(dst_odd, y1[:, :, :], RELU, bias=bias)

            # ---------------- Phase AB: p1 + p2 conv strips, bottom-up -----
            with (
                tc.tile_pool(name="w12", bufs=1) as w12,
                tc.tile_pool(name="xab", bufs=2) as xab,
                tc.tile_pool(name="dwp", bufs=2) as dwp,
                tc.tile_pool(name="p2s", bufs=3) as p2sp,
            ):
                wp1 = w12.tile([128, 24, 128], BF16, name="wp1t")
                wp2 = w12.tile([128, 24, 128], BF16, name="wp2t")
                nc.scalar.dma_start(out=wp1[:, 0:12, :], in_=wp1_d[0:12].rearrange("t i o -> i t o"))
                nc.scalar.dma_start(out=wp1[:, 12:24, :], in_=wp1_d[12:24].rearrange("t i o -> i t o"))
                nc.scalar.dma_start(out=wp2[:, 0:12, :], in_=wp2_d[0:12].rearrange("t i o -> i t o"))
                nc.scalar.dma_start(out=wp2[:, 12:24, :], in_=wp2_d[12:24].rearrange("t i o -> i t o"))

                def phase_c_add(s):
                    # s-add slice (disjoint across strips; includes the row
                    # above the strip so phase-C's X0 halo row is complete).
                    h0a = s * SH
                    alo = max(h0a - 1, 0)
                    ahi = h0a + SH - 1 if s < NS - 1 else H
                    p2l = p2lp.tile([128, SH + 1, W], BF16, name="p2l", tag="p2l")
                    nr = ahi - alo
                    nc.scalar.dma_start(
                        out=p2l[:, 0:nr, :], in_=pool2_d[:, alo:ahi, :]
                    )
                    nc.vector.tensor_tensor(
                        out=p1buf[:, 1 + alo : 1 + ahi, 1 : W + 1],
                        in0=p1buf[:, 1 + alo : 1 + ahi, 1 : W + 1],
                        in1=p2l[:, 0:nr, :],
                        op=ADD,
                    )

                for s in range(NS - 1, -1, -1):
                    if s == max(NS - 5, 0):
                        load_cd_weights()
                    h0 = s * SH
                    xt = []
                    for kb in range(2):
                        t = xab.tile(
                            [128, SH + 2, W + 2], BF16, name=f"xab{kb}", tag=f"xab{kb}"
                        )
                        nc.gpsimd.memset(t[:, :, 0:1], 0.0)
                        nc.gpsimd.memset(t[:, :, W + 1 : W + 2], 0.0)
                        glo = max(h0 - 1, 0)
                        ghi = min(h0 + SH + 1, H)
                        brow = glo - (h0 - 1)
                        # scalar queue: the baseline's proven DMA->DVE edge
                        # (sync-queue DMA -> DVE transform showed cold-start
                        # corruption on HW).
                        nc.scalar.dma_start(
                            out=t[:, brow : brow + (ghi - glo), 1 : W + 1],
                            in_=x_d[kb * 128 : (kb + 1) * 128, glo:ghi, :],
                        )
                        if s == 0:
                            nc.gpsimd.memset(t[:, 0:1, :], 0.0)
                        if s == NS - 1:
                            nc.gpsimd.memset(t[:, SH + 1 : SH + 2, :], 0.0)
                        xt.append(t)

                    # winograd row transform per kb
                    dw = []
                    for kb in range(2):
                        d = dwp.tile(
                            [128, 4, 4, W + 2], BF16, name=f"dw{kb}", tag=f"dw{kb}"
                        )
                        transform(d, xt[kb])
                        dw.append(d)

                    # p1 conv -> p1buf rows (stride-2 even/odd evictions)
                    ps = []
                    for m in range(4):
                        pst = psum_pool.tile([128, 4, W], F32, name=f"ps1_{m}", tag="ps")
                        n = 0
                        for kb in range(2):
                            for dx in range(3):
                                nc.tensor.matmul(
                                    pst[:, :, :],
                                    wp1[:, kb * 12 + dx * 4 + m, :],
                                    dw[kb][:, m, :, dx : dx + W],
                                    start=(n == 0),
                                    stop=(n == 5),
                                )
                                n += 1
                        ps.append(pst)
                    combine_evict(
                        ps,
                        p1buf[:, 1 + h0 : 9 + h0 : 2, 1 : W + 1],
                        p1buf[:, 2 + h0 : 10 + h0 : 2, 1 : W + 1],
                        bp1[:, 0:1],
                    )

                    # p2 conv -> strip tile, in-place W suffix-max, -> DRAM
                    p2t = p2sp.tile([128, SH, W], BF16, name="p2t", tag="p2t")
                    ps = []
                    for m in range(4):
                        pst = psum_pool.tile([128, 4, W], F32, name=f"ps2_{m}", tag="ps")
                        n = 0
                        for kb in range(2):
                            for dx in range(3):
                                nc.tensor.matmul(
                                    pst[:, :, :],
                                    wp2[:, kb * 12 + dx * 4 + m, :],
                                    dw[kb][:, m, :, dx : dx + W],
                                    start=(n == 0),
                                    stop=(n == 5),
                                )
                                n += 1
                        ps.append(pst)
                    combine_evict(
                        ps, p2t[:, 0:8:2, :], p2t[:, 1:8:2, :], bp2[:, 0:1]
                    )
                    d = 1
                    while d < W:
                        nc.vector.tensor_tensor(
                            out=p2t[:, :, 0 : W - d],
                            in0=p2t[:, :, 0 : W - d],
                            in1=p2t[:, :, d:W],
                            op=MAX,
                        )
                        d *= 2
                    nc.sync.dma_start(out=pool2_d[:, h0 : h0 + SH, :], in_=p2t[:, :, :])

                    # pool1 row chain for this strip (row h = max(row h, row h+1))
                    for h in range(min(h0 + SH - 1, H - 2), h0 - 1, -1):
                        nc.vector.tensor_tensor(
                            out=p1buf[:, 1 + h : 2 + h, 1 : W + 1],
                            in0=p1buf[:, 1 + h : 2 + h, 1 : W + 1],
                            in1=p1buf[:, 2 + h : 3 + h, 1 : W + 1],
                            op=MAX,
                        )
                    if s + 1 <= NS - 1:
                        phase_c_add(s + 1)
                phase_c_add(0)

            # ---------------- Phase C+D interleaved, bottom-up -------------
            with (
                tc.tile_pool(name="ost", bufs=3) as ost,
            ):
                # r ring: [mb][slot] padded strip buffers (4-deep: phase_d
                # lags phase_c by two strips so its halo+transform prep can
                # run during the preceding strip's matmuls)
                RING = 4
                rslot = [
                    [
                        rring.tile([128, SH + 2, W + 2], BF16, name=f"rs{mb}_{k}")
                        for k in range(RING)
                    ]
                    for mb in range(2)
                ]
                for mb in range(2):
                    for k in range(RING):
                        nc.gpsimd.memset(rslot[mb][k][:, :, 0:1], 0.0)
                        nc.gpsimd.memset(
                            rslot[mb][k][:, :, W + 1 : W + 2], 0.0
                        )

                def phase_c_prep(s):
                    # xc DMA + winograd transform of s = pool1+pool2 (p1buf
                    # rows h0..h0+9 == s-image rows h0-1..h0+8, pads included)
                    h0 = s * SH
                    xc = []
                    for kb in range(2):
                        t = xcp.tile([128, SH, W], BF16, name=f"xc{kb}", tag=f"xc{kb}")
                        nc.scalar.dma_start(
                            out=t[:, :, :],
                            in_=x_d[kb * 128 : (kb + 1) * 128, h0 : h0 + SH, :],
                        )
                        xc.append(t)
                    sw = swp.tile([128, 4, 4, W + 2], BF16, name="sw", tag="sw")
                    transform(sw, p1buf[:, h0 : h0 + SH + 2, :])
                    return xc, sw

                def phase_c_mm(s, prep):
                    h0 = s * SH
                    xc, sw = prep
                    for mb in range(2):
                        slot = rslot[mb][s % RING]
                        ps = []
                        for m in range(4):
                            pst = psum_pool.tile([128, 4, W], F32, name=f"psc_{m}", tag="ps")
                            n = 0
                            nmax = 4 if m in (0, 3) else 2
                            if m == 0:
                                for kb in range(2):
                                    nc.tensor.matmul(
                                        pst[:, :, :],
                                        wc1t[:, kb, mb * 128 : (mb + 1) * 128],
                                        xc[kb][:, 0:8:2, :],
                                        start=(n == 0),
                                        stop=False,
                                    )
                                    n += 1
                            if m == 3:
                                for kb in range(2):
                                    nc.tensor.matmul(
                                        pst[:, :, :],
                                        wc1t[:, 2 + kb, mb * 128 : (mb + 1) * 128],
                                        xc[kb][:, 1:8:2, :],
                                        start=(n == 0),
                                        stop=False,
                                    )
                                    n += 1
                            for dx in range(3):
                                nc.tensor.matmul(
                                    pst[:, :, :],
                                    wpt[:, dx * 4 + m, mb * 128 : (mb + 1) * 128],
                                    sw[:, m, :, dx : dx + W],
                                    start=(n == 0),
                                    stop=(n == nmax),
                                )
                                n += 1
                            ps.append(pst)
                        combine_evict(
                            ps,
                            slot[:, 1:9:2, 1 : W + 1],
                            slot[:, 2:10:2, 1 : W + 1],
                            bpc1[:, mb : mb + 1],
                        )

                def phase_d_prep(s):
                    # halo rows + winograd transform of r; depends only on
                    # phase_c(s-1..s+1), all finished >=1 iteration earlier.
                    for mb in range(2):
                        slot = rslot[mb][s % RING]
                        if s == NS - 1:
                            nc.gpsimd.memset(
                                slot[:, SH + 1 : SH + 2, :], 0.0
                            )
                        else:
                            nc.scalar.copy(
                                slot[:, SH + 1 : SH + 2, :],
                                rslot[mb][(s + 1) % RING][:, 1:2, :],
                            )
                        if s == 0:
                            nc.gpsimd.memset(slot[:, 0:1, :], 0.0)
                        else:
                            nc.scalar.copy(
                                slot[:, 0:1, :], rslot[mb][(s - 1) % RING][:, SH : SH + 1, :]
                            )
                    rw = []
                    for kb in range(2):
                        d = rwp.tile(
                            [128, 4, 4, W + 2], BF16, name=f"rw{kb}", tag=f"rw{kb}"
                        )
                        transform(d, rslot[kb][s % RING])
                        rw.append(d)
                    return rw

                def phase_d_mm(s, rw):
                    h0 = s * SH
                    for mb in range(2):
                        ps = []
                        for m in range(4):
                            pst = psum_pool.tile([128, 4, W], F32, name=f"psd_{m}", tag="ps")
                            n = 0
                            for kb in range(2):
                                for dx in range(3):
                                    nc.tensor.matmul(
                                        pst[:, :, :],
                                        wc2t[:, kb * 12 + dx * 4 + m, mb * 128 : (mb + 1) * 128],
                                        rw[kb][:, m, :, dx : dx + W],
                                        start=(n == 0),
                                        stop=(n == 5),
                                    )
                                    n += 1
                            ps.append(pst)
                        ot = ost.tile([128, SH, W], F32, name="otile", tag="otile")
                        combine_evict(
                            ps, ot[:, 0:8:2, :], ot[:, 1:8:2, :], bc2[:, mb : mb + 1]
                        )
                        nc.sync.dma_start(
                            out=out_d[mb * 128 : (mb + 1) * 128, h0 : h0 + SH, :],
                            in_=ot[:, :, :],
                        )

                # phase_d lags phase_c by 2 strips; per iteration, next-
                # iteration prep (halos/transforms/DMA) is emitted before the
                # matmul batches so PE never waits on it.
                prep_c = {NS - 1: phase_c_prep(NS - 1), NS - 2: phase_c_prep(NS - 2)}
                phase_c_mm(NS - 1, prep_c.pop(NS - 1))
                prep_c[NS - 3] = phase_c_prep(NS - 3)
                phase_c_mm(NS - 2, prep_c.pop(NS - 2))
                for s in range(NS - 3, -1, -1):
                    rw = phase_d_prep(s + 2)
                    if s >= 1:
                        prep_c[s - 1] = phase_c_prep(s - 1)
                    phase_c_mm(s, prep_c.pop(s))
                    phase_d_mm(s + 2, rw)
                rw = phase_d_prep(1)
                phase_d_mm(1, rw)
                rw = phase_d_prep(0)
                phase_d_mm(0, rw)

    split_excess_sync(nc)
    return nc


# ---------------------------------------------------------------------------
def _fold(Wc, g, b, m, v):
    scale = (g / np.sqrt(v + EPS)).astype(np.float64)
    Wf = Wc.astype(np.float64) * scale[:, None, None, None]
    bias = b.astype(np.float64) - m.astype(np.float64) * scale
    return Wf, bias.astype(np.float32)


def _pack_wg(Wf):
    # Wf: [O, I, 3, 3] float64 -> [n_kb*12 (kb,dx,m), 128, O] bf16
    O, I = Wf.shape[:2]
    n_kb = I // 128
    out = np.empty((n_kb * 12, 128, O), dtype=NP_BF16)
    for kb in range(n_kb):
        blk = Wf[:, kb * 128 : (kb + 1) * 128]  # [O, 128, 3, 3]
        for dx in range(3):
            w0, w1, w2 = blk[:, :, 0, dx], blk[:, :, 1, dx], blk[:, :, 2, dx]
            wm = [w0, (w0 + w1 + w2) / 2, (w0 - w1 + w2) / 2, w2]
            for m in range(4):
                out[kb * 12 + dx * 4 + m] = wm[m].T.astype(NP_BF16)
    return out


def _prep_weights(inp):
    wp1f, bp1 = _fold(inp["W_p1"], inp["g_p1"], inp["b_p1"], inp["m_p1"], inp["v_p1"])
    wp2f, bp2 = _fold(inp["W_p2"], inp["g_p2"], inp["b_p2"], inp["m_p2"], inp["v_p2"])
    wpf, bp = _fold(inp["W_p"], inp["g_p"], inp["b_p"], inp["m_p"], inp["v_p"])
    wc1f, bc1 = _fold(inp["W_c1"], inp["g_c1"], inp["b_c1"], inp["m_c1"], inp["v_c1"])
    wc2f, bc2 = _fold(inp["W_c2"], inp["g_c2"], inp["b_c2"], inp["m_c2"], inp["v_c2"])
    wc1_pos = [wc1f[:, kb * 128 : (kb + 1) * 128, 0, 0].T for kb in range(2)]
    wc1_all = np.stack(wc1_pos + [-w for w in wc1_pos]).astype(NP_BF16)
    return {
        "wp1": _pack_wg(wp1f),
        "wp2": _pack_wg(wp2f),
        "wp": _pack_wg(wpf),
        "wc2": _pack_wg(wc2f),
        "wc1": wc1_all,
        "bp1": bp1.astype(np.float32).reshape(128, 1),
        "bp2": bp2.astype(np.float32).reshape(128, 1),
        "bpc1": (bp + bc1).astype(np.float32).reshape(2, 128).T.copy(),
        "bc2": bc2.astype(np.float32).reshape(2, 128).T.copy(),
    }


_nc_cache = {}


def _get_nc(H):
    if H not in _nc_cache:
        _nc_cache[H] = build_nc(H)
    return _nc_cache[H]


def run(inputs, H=128, trace=False):
    nc = _get_nc(H)
    inputs = {k: np.asarray(v) for k, v in inputs.items()}
    wd = _prep_weights(inputs)
    x = np.asarray(inputs["x"], dtype=np.float32).astype(NP_BF16)
    B = x.shape[0]
    in_maps = [dict(wd, x=np.ascontiguousarray(x[i, :, :H, :])) for i in range(B)]
    res = run_bass_kernel_spmd(nc, in_maps, core_ids=list(range(B)), trace=trace)
    out = np.stack([res.results[i]["out"] for i in range(B)])
    return out, res


def kernel(**inputs):
    out, _ = run(inputs, H=128, trace=False)
    return out
